# revision 11
# baseline (speedup 1.0000x reference)
"""Single-head attention (InterModalAttention) Bass kernel for 8 TRN2 cores.

Sharding: batch (4) x seq-half (2) -> 8 cores. Core (2b+h) projects Q/K/V
for its OWN 1024 rows of batch b. K and V are exchanged within the pair
(2b, 2b+1) via pairwise AllReduce(add) of the own half (K in 1 piece,
V in 2); each core recovers the peer half as peer = sum - own (DVE for
K -> fp8, gpsimd for V -> f16). The own half never leaves SBUF, and the
key layout is rank-symmetric: every core processes keys in the order
[own 1024, peer 1024]. Attention is permutation-invariant over keys, so
own-key score/output tiles have no collective dependency and the
collective latency hides behind them.

Precision (numpy sim rel-err 1.58e-2 vs the 2e-2 gate; HW matches sim):
  - fp16 for projections and attn@V (same PE rate as bf16, 8x lower
    quantization error); fp32 accumulation in PSUM; f16 exchange.
  - scores matmul in fp8-e4m3 perf_mode=DoubleRow: contracts 256/MM,
    halving score matmul count. The [P, et, cols] layout keeps et-pairs
    adjacent so DoubleRow's [Ki, 2, free] AP falls out directly.

Bias-via-matmul: the Q bias and the output bias are folded into the
PSUM accumulations as K=1 matmuls (bq16.T @ ones_row, rs16.T @ bv16),
so the Q epilogue is a pure ACT fp8 convert (no DVE on the scores-start
critical path) and the output epilogue is ACT-scale -> DMA only.

DMA plan: inputs are pre-transposed on the host into exact SBUF layouts
(4-8KB contiguous runs per partition) AND split into ~0.5MB pieces so
both hardware queues keep several rings busy. The sync engine's queue
drains by ~7us so the exchange bounces never wait behind input loads;
scalar carries the long tail of loads. All exchange DMAs sit on sync in
completion order; every per-engine FIFO is monotone in data-ready time.
"""
import sys
import numpy as np

for p in ("/opt/trn_rl_repo",):
    if p not in sys.path:
        sys.path.insert(0, p)

B, S, D = 4, 2048, 1024
NQ = 1024          # queries (and own keys) per core
NCORES = 8
P = 128
INV_SQRT_D = 1.0 / 32.0
PAIRS = [[0, 1], [2, 3], [4, 5], [6, 7]]

_CACHE = {}


def build_nc():
    from contextlib import ExitStack
    import concourse.mybir as mybir
    import concourse.tile as tile
    from concourse import bacc

    F32 = mybir.dt.float32
    F16 = mybir.dt.float16
    F8 = mybir.dt.float8e4
    AF = mybir.ActivationFunctionType
    DR = mybir.MatmulPerfMode.DoubleRow
    SUB = mybir.AluOpType.subtract

    nc = bacc.Bacc("TRN2", debug=False, num_devices=NCORES)

    ET = D // P            # 8 e-tiles
    DT = D // P            # 8 d-tiles
    HC = NQ // 512         # 2 s-chunks over own half
    SB = S // P            # 16 j-tiles (per-core order: 0-7 own, 8-15 peer)
    HB = NQ // P           # 8 j-tiles (own half)
    IG = NQ // 512         # 2 i-chunks
    EC = D // 512          # 2 e-chunks
    ETH = ET // 2          # 4 et-pairs for DoubleRow

    # inputs pre-transposed on host into SBUF layouts
    x2 = nc.dram_tensor("x2", (HC, P, DT, 512), F16, kind="ExternalInput")
    wq2 = nc.dram_tensor("wq2", (P, DT, D), F16, kind="ExternalInput")
    wk2 = nc.dram_tensor("wk2", (P, DT, D), F16, kind="ExternalInput")
    wv2 = nc.dram_tensor("wv2", (P, DT, D), F16, kind="ExternalInput")
    bq = nc.dram_tensor("bq", (D,), F32, kind="ExternalInput")
    bk = nc.dram_tensor("bk", (D,), F32, kind="ExternalInput")
    bv = nc.dram_tensor("bv", (D,), F32, kind="ExternalInput")
    out = nc.dram_tensor("out", (NQ, D), F32, kind="ExternalOutput")

    with tile.TileContext(nc) as tc, ExitStack() as ctx:
        consts = ctx.enter_context(tc.tile_pool(name="consts", bufs=1))

        # resident tensors
        kqv = ctx.enter_context(tc.tile_pool(name="kqv", bufs=1))
        kT8 = kqv.tile([P, ET, S], F8)       # [d-part, e-tile, key] own|peer
        qT8 = kqv.tile([P, ET, NQ], F8)      # [d-part, e-tile, i]
        vN = kqv.tile([P, SB, D], F16)       # [j-part, j-tile, e] own|peer
        kf16 = kqv.tile([P, ET, NQ], F16)    # own K, f16 (bounce + subtract)
        ksum = kqv.tile([P, ET, NQ], F16)    # pair sum of K
        vsum = kqv.tile([P, HB, D], F16)     # pair sum of V

        # DRAM buffers for the pairwise K/V AllReduce
        ccd = ctx.enter_context(tc.tile_pool(name="ccd", bufs=1, space="DRAM"))
        kb_in = ccd.tile([P, ET, NQ], F16)
        kb_out = ccd.tile([P, ET, NQ], F16)
        vb_in = [ccd.tile([P, 4, D], F16, tag=f"vbi{c}", name=f"vbi{c}")
                 for c in range(HC)]
        vb_out = [ccd.tile([P, 4, D], F16, tag=f"vbo{c}", name=f"vbo{c}")
                  for c in range(HC)]

        # pp spans K1/V/Q projections AND scores; closed before outps.
        pp_stack = ExitStack()

        # ---- Phase 1: projections over own half, single pass over x ----
        with tc.tile_pool(name="w", bufs=1) as wp, \
             tc.tile_pool(name="xc", bufs=2) as xcp:
            wk_sb = wp.tile([P, DT, D], F16)
            wq_sb = wp.tile([P, DT, D], F16)
            wv_sb = wp.tile([P, DT, D], F16)
            xc = []
            for hc in range(HC):
                xc.append(xcp.tile([P, DT, 512], F16, tag="xc", name=f"xc{hc}"))

            # biases + consts first (tiny). bq/bv as f16 rows for the
            # bias-via-matmul trick; bk as per-partition f32 for DVE adds.
            bk_sb = consts.tile([P, ET], F32)
            nc.scalar.dma_start(bk_sb[:], bk[:].rearrange("(t p) -> p t", p=P))
            bq16 = consts.tile([1, D], F16)
            nc.gpsimd.dma_start(bq16[:], bq[:].rearrange("(one d) -> one d", one=1))
            bv16 = consts.tile([1, D], F16)
            nc.gpsimd.dma_start(bv16[:], bv[:].rearrange("(one d) -> one d", one=1))
            ones_row = consts.tile([1, 512], F16)
            nc.vector.memset(ones_row[:], 1.0)
            onesb = consts.tile([P, 1], F32)
            nc.vector.memset(onesb[:], 1.0)
            rs16s = [consts.tile([1, 512], F16, tag=f"rs16_{g}", name=f"rs16_{g}")
                     for g in range(IG)]

            # big loads: consumption order (x0,wk) -> x1 -> wv -> wq in
            # ~0.5MB pieces. sync's queue drains by ~7us so the exchange
            # bounces below never wait; scalar carries the long tail.
            nc.sync.dma_start(xc[0][:, 0:4, :], x2[0, :, 0:4, :])
            nc.scalar.dma_start(wk_sb[:, 0:2, :], wk2[:, 0:2, :])
            nc.sync.dma_start(xc[0][:, 4:8, :], x2[0, :, 4:8, :])
            nc.scalar.dma_start(wk_sb[:, 2:4, :], wk2[:, 2:4, :])
            nc.sync.dma_start(wk_sb[:, 4:6, :], wk2[:, 4:6, :])
            nc.scalar.dma_start(wk_sb[:, 6:8, :], wk2[:, 6:8, :])
            nc.sync.dma_start(xc[1][:, 0:4, :], x2[1, :, 0:4, :])
            nc.scalar.dma_start(xc[1][:, 4:8, :], x2[1, :, 4:8, :])
            for i in range(4):
                nc.scalar.dma_start(wv_sb[:, 2 * i:2 * i + 2, :],
                                    wv2[:, 2 * i:2 * i + 2, :])
            for i in range(4):
                nc.scalar.dma_start(wq_sb[:, 2 * i:2 * i + 2, :],
                                    wq2[:, 2 * i:2 * i + 2, :])

            # K chunk 0: dt-outer/et-inner over 8 PSUM banks so the PE
            # starts as soon as the first x/weight pieces land.
            with tc.tile_pool(name="p8", bufs=8, space="PSUM") as p8:
                psk0 = [p8.tile([P, 512], F32, tag="p8", name=f"psk0_{et}")
                        for et in range(ET)]
                for dt in range(DT):
                    for et in range(ET):
                        nc.tensor.matmul(psk0[et][:],
                                         wk_sb[:, dt, et * P:(et + 1) * P],
                                         xc[0][:, dt, :], start=(dt == 0),
                                         stop=(dt == DT - 1))
                for et in range(ET):
                    nc.vector.tensor_scalar_add(kf16[:, et, 0:512],
                                                psk0[et][:], bk_sb[:, et:et + 1])
                    nc.scalar.activation(kT8[:, et, 0:512], kf16[:, et, 0:512],
                                         AF.Copy)

            pp = pp_stack.enter_context(
                tc.tile_pool(name="pp", bufs=4, space="PSUM"))
            # K chunk 1
            for et in range(ET):
                psk = pp.tile([P, 512], F32, tag="pp")
                for dt in range(DT):
                    nc.tensor.matmul(psk[:], wk_sb[:, dt, et * P:(et + 1) * P],
                                     xc[1][:, dt, :], start=(dt == 0),
                                     stop=(dt == DT - 1))
                nc.vector.tensor_scalar_add(kf16[:, et, 512:1024],
                                            psk[:], bk_sb[:, et:et + 1])
                nc.scalar.activation(kT8[:, et, 512:1024], kf16[:, et, 512:1024],
                                     AF.Copy)
            # own K complete -> bounce + AllReduce (one 2MB piece)
            nc.sync.dma_start(kb_in[:], kf16[:])
            nc.gpsimd.collective_compute(
                "AllReduce", mybir.AluOpType.add, replica_groups=PAIRS,
                ins=[kb_in[:].opt()], outs=[kb_out[:].opt()])

            # V projection (own half) -> vN j-tiles 0..7 (copies on ACT);
            # bounce + AllReduce per 4-tile half
            for hc in range(HC):
                for sb_i in range(4):
                    jg = hc * 4 + sb_i
                    for ec in range(EC):
                        psv = pp.tile([P, 512], F32, tag="pp")
                        for dt in range(DT):
                            nc.tensor.matmul(psv[:],
                                             xc[hc][:, dt, sb_i * P:(sb_i + 1) * P],
                                             wv_sb[:, dt, ec * 512:(ec + 1) * 512],
                                             start=(dt == 0), stop=(dt == DT - 1))
                        nc.scalar.activation(
                            vN[:, jg, ec * 512:(ec + 1) * 512], psv[:], AF.Copy)
                nc.sync.dma_start(vb_in[hc][:], vN[:, hc * 4:(hc + 1) * 4, :])
                nc.gpsimd.collective_compute(
                    "AllReduce", mybir.AluOpType.add, replica_groups=PAIRS,
                    ins=[vb_in[hc][:].opt()], outs=[vb_out[hc][:].opt()])

            # sum-ins on sync, completion order
            nc.sync.dma_start(ksum[:, :, 0:512], kb_out[:, :, 0:512])
            nc.sync.dma_start(ksum[:, :, 512:1024], kb_out[:, :, 512:1024])
            for half in range(2):
                nc.sync.dma_start(vsum[:, half * 4:(half + 1) * 4, :],
                                  vb_out[half][:])

            # Q projection -> fp8; bias folded in as a K=1 matmul so the
            # epilogue is a pure ACT convert (DVE only does K subtracts)
            for et in range(ET):
                nc.vector.tensor_tensor(kT8[:, et, NQ:NQ + 512],
                                        ksum[:, et, 0:512],
                                        kf16[:, et, 0:512], op=SUB)
            for hc in range(HC):
                for et in range(ET):
                    psq = pp.tile([P, 512], F32, tag="pp")
                    for dt in range(DT):
                        nc.tensor.matmul(psq[:],
                                         wq_sb[:, dt, et * P:(et + 1) * P],
                                         xc[hc][:, dt, :], start=(dt == 0),
                                         stop=False)
                    nc.tensor.matmul(psq[:], bq16[:, et * P:(et + 1) * P],
                                     ones_row[:], start=False, stop=True)
                    nc.scalar.activation(
                        qT8[:, et, hc * 512:(hc + 1) * 512], psq[:], AF.Copy)
            for et in range(ET):
                nc.vector.tensor_tensor(kT8[:, et, NQ + 512:S],
                                        ksum[:, et, 512:1024],
                                        kf16[:, et, 512:1024], op=SUB)
            # V subtracts on gpsimd (f16), split by vsum-in half
            for jg in range(HB):
                nc.gpsimd.tensor_tensor(vN[:, HB + jg, :], vsum[:, jg, :],
                                        vN[:, jg, :], op=SUB)

        # ---- Phase 2: scores (fp8 DoubleRow) then output matmuls ----
        with tc.tile_pool(name="attn", bufs=1) as attnp, \
             tc.tile_pool(name="epi2", bufs=2) as epi2p, \
             tc.tile_pool(name="rsdram", bufs=2, space="DRAM") as rsdram, \
             tc.tile_pool(name="epi", bufs=2) as epip:
            attnTs = [attnp.tile([P, SB, 512], F16, tag=f"attnT{g}", name=f"attnT{g}")
                      for g in range(IG)]
            accs = [epip.tile([P, 512], F32, tag="acc", name=f"acc{g}")
                    for g in range(IG)]
            invss = [epi2p.tile([P, 4], F32, tag="invs", name=f"invs{g}")
                     for g in range(IG)]

            def scores_tile(g, jt):
                attnT = attnTs[g]
                sc_ps = pp.tile([P, 512], F32, tag="pp")
                for t in range(ETH):
                    nc.tensor.matmul(
                        sc_ps[:],
                        kT8[:, 2 * t:2 * t + 2, jt * P:(jt + 1) * P],
                        qT8[:, 2 * t:2 * t + 2, g * 512:(g + 1) * 512],
                        start=(t == 0), stop=(t == ETH - 1),
                        perf_mode=DR)
                nc.scalar.activation(attnT[:, jt, :], sc_ps[:], AF.Exp,
                                     scale=INV_SQRT_D)
                if jt == 0:
                    nc.vector.tensor_copy(accs[g][:], attnT[:, 0, :])
                else:
                    nc.vector.tensor_add(accs[g][:], accs[g][:], attnT[:, jt, :])

            def rowsum(g, psum_pool):
                # ones-matmul partition sum -> DRAM-bounce transpose -> 1/x
                rs = psum_pool.tile([1, 512], F32, tag="rs", name=f"rs{g}",
                                    bufs=1)
                nc.tensor.matmul(rs[:], onesb[:], accs[g][:], start=True,
                                 stop=True)
                rs_sb = epip.tile([1, 512], F32, tag="rs_sb")
                nc.vector.tensor_copy(rs_sb[:], rs[:])
                nc.scalar.activation(rs16s[g][:], rs_sb[:], AF.Copy)
                rs_d = rsdram.tile([1, 512], F32, tag="rs_d")
                nc.sync.dma_start(rs_d[:], rs_sb[:])
                rsT = epip.tile([P, 4], F32, tag="rsT")
                nc.sync.dma_start(
                    rsT[:], rs_d[:].rearrange("one (b p) -> p (one b)", p=P))
                nc.vector.reciprocal(invss[g][:], rsT[:])

            def out_block(g, ib, outps, nsl, last=False):
                # out_psum = sum_j attn_j @ v_j + rs.T @ bv  (K=1 matmul),
                # so out = psum * (1/rs) needs no separate bias add.
                attnT = attnTs[g]
                i0 = ib * P
                ops = [outps.tile([P, 512], F32, tag=f"outps{ec}",
                                  name=f"ops{g}_{ib}_{ec}") for ec in range(EC)]
                for jt in range(SB):
                    for ec in range(EC):
                        nc.tensor.matmul(ops[ec][:],
                                         attnT[:, jt, i0:i0 + P],
                                         vN[:, jt, ec * 512:(ec + 1) * 512],
                                         start=(jt == 0), stop=False)
                for ec in range(EC):
                    nc.tensor.matmul(ops[ec][:],
                                     rs16s[g][:, i0:i0 + P],
                                     bv16[:, ec * 512:(ec + 1) * 512],
                                     start=False, stop=True)
                out_sb = epi2p.tile([P, D], F32, tag="out_sb")
                r0 = g * 512 + i0
                w = D // nsl
                for sl_i in range(nsl):
                    sl = slice(sl_i * w, (sl_i + 1) * w)
                    ec = (sl_i * w) // 512
                    psl = slice(sl_i * w - ec * 512, (sl_i + 1) * w - ec * 512)
                    nc.scalar.activation(out_sb[:, sl], ops[ec][:, psl],
                                         AF.Copy, scale=invss[g][:, ib:ib + 1])
                    # last block drains via two queues in parallel
                    eng = nc.gpsimd if (last and sl_i < nsl // 2) else nc.sync
                    eng.dma_start(out[r0:r0 + P, sl], out_sb[:, sl])

            for jt in range(SB):
                scores_tile(0, jt)
            scores_tile(1, 0)
            scores_tile(1, 1)
            rowsum(0, pp)       # acc0 is complete; PE is busy in g1 tiles
            for jt in range(2, SB):
                scores_tile(1, jt)
            pp_stack.close()
            with tc.tile_pool(name="outps", bufs=3, space="PSUM") as outps:
                out_block(0, 0, outps, 2)
                rowsum(1, outps)    # acc1 complete; PE busy in output block
                out_block(0, 1, outps, 2)
                out_block(0, 2, outps, 2)
                out_block(0, 3, outps, 2)
                out_block(1, 0, outps, 2)
                out_block(1, 1, outps, 2)
                out_block(1, 2, outps, 4)
                out_block(1, 3, outps, 4, last=True)

    nc.compile()
    return nc


def make_in_maps(x, Wq, bq, Wk, bk, Wv, bv):
    F16 = np.float16
    x = np.asarray(x, np.float32)

    def wprep(W):
        # [P, DT, D] with [p, t, d] = W.T[t*128+p, d]
        wT = np.asarray(W, np.float32).T.astype(F16)
        return np.ascontiguousarray(wT.reshape(8, P, D).transpose(1, 0, 2))

    wq2, wk2, wv2 = wprep(Wq), wprep(Wk), wprep(Wv)
    bq = np.ascontiguousarray(np.asarray(bq, np.float32))
    bk = np.ascontiguousarray(np.asarray(bk, np.float32))
    bv = np.ascontiguousarray(np.asarray(bv, np.float32))
    in_maps = []
    for c in range(NCORES):
        b, h = c // 2, c % 2
        xT = x[b][h * NQ:(h + 1) * NQ].T.astype(F16)     # [D, NQ]
        # [HC, P, DT, 512] with [hc, p, t, j] = xT[t*128+p, hc*512+j]
        x2 = xT.reshape(8, P, 2, 512).transpose(2, 1, 0, 3)
        in_maps.append({
            "x2": np.ascontiguousarray(x2),
            "wq2": wq2, "wk2": wk2, "wv2": wv2,
            "bq": bq, "bk": bk, "bv": bv,
        })
    return in_maps


def get_nc():
    if "nc" not in _CACHE:
        _CACHE["nc"] = build_nc()
    return _CACHE["nc"]


def kernel(x, Wq, bq, Wk, bk, Wv, bv):
    from concourse.bass_utils import run_bass_kernel_spmd
    nc = get_nc()
    in_maps = make_in_maps(x, Wq, bq, Wk, bk, Wv, bv)
    res = run_bass_kernel_spmd(nc, in_maps, core_ids=list(range(NCORES)))
    out = np.empty((B, S, D), np.float32)
    for c in range(NCORES):
        b, h = c // 2, c % 2
        out[b, h * NQ:(h + 1) * NQ] = res.results[c]["out"]
    return out
